# Initial kernel scaffold
#
"""Trainium2 Bass kernel for the ConsistencyLoss problem (v4).

Inputs: semantic_pred (B,N) int32, instance_masks (B,M,N) f32, depth (B,N) f32
with B=16, M=32, N=65536 (H=W=256), C=27 classes. Outputs the scalar tuple
(l_uniform, l_boundary, l_dbc, total).

Sharding: pure data-parallel over batch; 2 batches per core on 8 cores. Each
core emits 10 partial sums (5 per batch); the host combines the 4 scalars.

Per-core pipeline (per batch):
  * histogram: masks stream in as [128=(g,m), 2048] f32 tiles; each tile is
    four [32, 2048] quarter-DMAs spread over sync/scalar/vector/gpsimd so
    both HWDGE rings plus SWDGE queues run in parallel. Tiles are converted
    to bf16 on ACT, then pair-transposed on the DVE by bitcasting bf16
    pairs to u32 (halves DVE transpose cost). Each 64-col stationary block
    covers 256 pixels; one matmul per block with a packed even/odd one-hot
    moving operand (N=54) accumulates an interleaved histogram in PSUM.
    Constant parity-selection matmuls fold the PSUM into hist[m, c].
  * the one-hot is built densely in one DVE 2x tensor_tensor per tile from
    the pair-transposed sem tile: m2[p, c*64+2kb+e] = (semT[p,2kb+e] == c).
  * boundary/depth losses: (256,256) views as [128, (2,256)] tiles; shifted
    rows derived on-chip via SBUF-to-SBUF DMA; emitted between the two
    batch streams so the work fills engine gaps; cross-partition fold via
    a ones matmul.
"""

import os

os.environ.setdefault("MYCRO_LOCAL_CACHE", "1")

import numpy as np
from contextlib import ExitStack

B, M, N, C = 16, 32, 65536, 27
H = W = 256
NCORES = 8
BPC = B // NCORES          # batches per core
ST = 8                     # super-tiles per batch
G = 4                      # mask-partition groups of 32
FS = 2048                  # pixels per group per super-tile (8192/tile)
KB = 32                    # stationary k-blocks per tile (256 px each)
NSTAT = 10                 # 5 partial sums x 2 batches

LAST_EXEC_NS = None

_compiled = None


def _build():
    import concourse.tile as tile
    from concourse import bacc, mybir

    f32 = mybir.dt.float32
    i32 = mybir.dt.int32
    bf16 = mybir.dt.bfloat16
    OP = mybir.AluOpType
    AX = mybir.AxisListType
    AF = mybir.ActivationFunctionType

    nc = bacc.Bacc("TRN2", target_bir_lowering=False, debug=False,
                   enable_asserts=False, num_swdge_queues=4)
    sem_d = nc.dram_tensor("sem", [BPC, N], i32, kind="ExternalInput")
    masks_d = nc.dram_tensor("masks", [BPC, M, N], f32, kind="ExternalInput")
    depth_d = nc.dram_tensor("depth", [BPC, N], f32, kind="ExternalInput")
    out_d = nc.dram_tensor("partials", [NSTAT], f32, kind="ExternalOutput")

    with tile.TileContext(nc) as tc, ExitStack() as ctx:
        pconst = ctx.enter_context(tc.tile_pool(name="const", bufs=1))
        pin = ctx.enter_context(tc.tile_pool(name="maskin", bufs=4))
        pinb = ctx.enter_context(tc.tile_pool(name="maskinb", bufs=3))
        ptr = ctx.enter_context(tc.tile_pool(name="maskT", bufs=3))
        poh = ctx.enter_context(tc.tile_pool(name="ohp", bufs=3))
        psem = ctx.enter_context(tc.tile_pool(name="semp", bufs=2))
        pbnd = ctx.enter_context(tc.tile_pool(name="bnd", bufs=1))
        psm = ctx.enter_context(tc.tile_pool(name="small", bufs=2))
        pps = ctx.enter_context(tc.tile_pool(name="psum", bufs=1, space="PSUM"))

        do_hist = not bool(int(os.environ.get("KERNEL_SKIP_HIST", "0")))

        # ---- sem path first: it gates the first one-hot build ----
        # s_t[p=(g,jj), st*64 + 2kb + e] = sem[st*8192 + g*2048 + 64kb + 2jj + e]
        sT = []
        for b in range(BPC):
            s_in = psem.tile([128, ST * 64], i32, tag="s_in", name="s_in")
            nc.sync.dma_start(s_in[:], sem_d.ap()[b].rearrange(
                "(st g j f) -> (g j) st f", st=ST, g=G, j=32, f=64))
            s_bf = psem.tile([128, ST * 64], bf16, tag="s_bf", name="s_bf")
            nc.scalar.copy(s_bf[:], s_in[:])
            s_t = psem.tile([128, ST * 64], bf16, tag="s_t", name="s_t")
            nc.vector.transpose(s_t[:].bitcast(i32), s_bf[:].bitcast(i32))
            sT.append(s_t)

        # one-hot compare pattern: col (c, kb, e) -> value c
        iotarep_i = pconst.tile([128, 2 * C * KB], i32, tag="iotarep_i")
        nc.gpsimd.iota(iotarep_i[:], pattern=[[1, C], [0, KB], [0, 2]],
                       base=0, channel_multiplier=0)
        iotarep = pconst.tile([128, 2 * C * KB], bf16, tag="iotarep")
        nc.scalar.copy(iotarep[:], iotarep_i[:])

        stats = pconst.tile([128, NSTAT], f32, tag="stats")
        nc.vector.memset(stats[:], 0.0)
        ones = pconst.tile([128, 1], f32, tag="ones")
        nc.vector.memset(ones[:], 1.0)
        bias_ln = pconst.tile([M, 1], f32, tag="bias_ln")
        nc.vector.memset(bias_ln[:], 1e-10)
        bias_sq = pconst.tile([128, 1], f32, tag="bias_sq")
        nc.vector.memset(bias_sq[:], 1e-24)
        zerob = pconst.tile([128, 1], f32, tag="zerob")
        nc.vector.memset(zerob[:], 0.0)

        hist_ps = [pps.tile([64, 2 * C], f32, tag=f"hist{b}", name=f"hist{b}")
                   for b in range(BPC)]
        dma_engs = [nc.sync, nc.scalar, nc.gpsimd]

        def emit_hist_batch(b):
            for st in range(ST):
                tin = pin.tile([128, FS], f32, tag="tin", name="tin")
                src = masks_d.ap()[b].rearrange(
                    "m (st g f) -> st g m f", st=ST, g=G)
                for g in range(G):
                    dma_engs[(g + st) % 3].dma_start(
                        tin[32 * g:32 * (g + 1), :], src[st, g])
                tinb = pinb.tile([128, FS], bf16, tag="tinb", name="tinb")
                nc.scalar.copy(tinb[:], tin[:])
                tT = ptr.tile([128, FS], bf16, tag="tT", name="tT")
                nc.vector.transpose(tT[:].bitcast(i32), tinb[:].bitcast(i32))

                # one-hot: m2[p, c*64 + 2kb + e] = (s_t[p, st*64 + 2kb+e] == c)
                m2 = poh.tile([128, 2 * C * KB], bf16, tag="m2", name="m2")
                nc.vector.tensor_tensor(
                    m2[:].rearrange("p (c kb e) -> p c kb e", c=C, kb=KB),
                    sT[b][:, st * 64:(st + 1) * 64]
                        .rearrange("p (kb e) -> p kb e", kb=KB)
                        .unsqueeze(1).broadcast_to([128, C, KB, 2]),
                    iotarep[:].rearrange("p (c kb e) -> p c kb e", c=C, kb=KB),
                    op=OP.is_equal)

                mov = m2[:].rearrange("p (c kb e) -> p kb c e", c=C, kb=KB)
                for kb in range(KB):
                    nc.tensor.matmul(
                        hist_ps[b][:],
                        tT[:, 64 * kb:64 * (kb + 1)],
                        mov[:, kb],
                        start=(st == 0 and kb == 0),
                        stop=(st == ST - 1 and kb == KB - 1),
                    )

        def emit_entropy_epilogue(b):
            # hist[m, c] = psum[2m, 2c] + psum[2m+1, 2c+1]
            psum_sb = psm.tile([64, 2 * C], f32, tag="psum_sb", name="psum_sb")
            nc.scalar.copy(psum_sb[:], hist_ps[b][:])
            psv = psum_sb[:].rearrange("p (c e) -> p e c", e=2)
            h_ps = pps.tile([M, C], f32, tag=f"hps{b}", name=f"hps{b}")
            nc.tensor.matmul(h_ps[:], sele[:], psv[:, 0], start=True, stop=False)
            nc.tensor.matmul(h_ps[:], selo[:], psv[:, 1], start=False, stop=True)
            hist = psm.tile([M, C], f32, tag="hist_sb", name="hist_sb")
            nc.scalar.copy(hist[:], h_ps[:])
            ms0 = psm.tile([M, 1], f32, tag="ms0", name="ms0")
            nc.vector.tensor_reduce(ms0[:], hist[:], axis=AX.X, op=OP.add)
            ms = psm.tile([M, 1], f32, tag="ms", name="ms")
            nc.vector.tensor_scalar(ms[:], ms0[:], 1e-6, None, op0=OP.add)
            rec = psm.tile([M, 1], f32, tag="rec", name="rec")
            nc.vector.reciprocal(rec[:], ms[:])
            pr = psm.tile([M, C], f32, tag="pr", name="pr")
            nc.vector.tensor_scalar(pr[:], hist[:], rec[:, 0:1], None, op0=OP.mult)
            ql = psm.tile([M, C], f32, tag="ql", name="ql")
            nc.scalar.activation(ql[:], pr[:], AF.Ln, bias=bias_ln[0:M, 0:1])
            escr = psm.tile([M, C], f32, tag="escr", name="escr")
            nc.vector.tensor_tensor(escr[:], pr[:], ql[:], op=OP.mult)
            ent = psm.tile([M, 1], f32, tag="ent", name="ent")
            nc.vector.tensor_reduce(ent[:], escr[:], axis=AX.X, op=OP.add)
            nc.vector.tensor_scalar(stats[0:M, 5 * b + 4:5 * b + 5], ent[:],
                                    -1.0, None, op0=OP.mult)

        # ---- batch 0 histogram ----
        if do_hist:
            emit_hist_batch(0)

        # parity-selection matrices: sele[p, m] = (p == 2m), selo: (p == 2m+1)
        selrow = pconst.tile([64, M], i32, tag="selrow")
        nc.gpsimd.iota(selrow[:], pattern=[[0, M]], base=0, channel_multiplier=1)
        selc2 = pconst.tile([64, M], i32, tag="selc2")
        nc.gpsimd.iota(selc2[:], pattern=[[2, M]], base=0, channel_multiplier=0)
        selc2p = pconst.tile([64, M], i32, tag="selc2p")
        nc.gpsimd.iota(selc2p[:], pattern=[[2, M]], base=1, channel_multiplier=0)
        sele = pconst.tile([64, M], f32, tag="sele")
        nc.vector.tensor_tensor(sele[:], selrow[:], selc2[:], op=OP.is_equal)
        selo = pconst.tile([64, M], f32, tag="selo")
        nc.vector.tensor_tensor(selo[:], selrow[:], selc2p[:], op=OP.is_equal)

        if do_hist:
            emit_entropy_epilogue(0)

        # ---- boundary + depth losses (fills gaps during batch 1) ----
        def btile(tag, dt=f32):
            return pbnd.tile([128, BPC * 512], dt, tag=tag, name=tag)

        semn = btile("semn", i32)
        sems = btile("sems", i32)
        m0n = btile("m0n")
        m0s = btile("m0s")
        dn = btile("dn")
        ds_ = btile("ds_")

        for b in range(BPC):
            o = b * 512
            img = dict(t=2, p=128, w=W)
            nc.sync.dma_start(semn[:, o:o + 512],
                              sem_d.ap()[b].rearrange("(t p w) -> p t w", **img))
            nc.scalar.dma_start(m0n[:, o:o + 512],
                                masks_d.ap()[b, 0].rearrange("(t p w) -> p t w", **img))
            nc.sync.dma_start(dn[:, o:o + 512],
                              depth_d.ap()[b].rearrange("(t p w) -> p t w", **img))

        # shifted-by-one-image-row tiles, derived on-chip
        for nat, sh in ((semn, sems), (m0n, m0s), (dn, ds_)):
            for b in range(BPC):
                o = b * 512
                nc.sync.dma_start(sh[1:128, o:o + 512], nat[0:127, o:o + 512])
                nc.sync.dma_start(sh[0:1, o + 256:o + 512], nat[127:128, o:o + 256])
                nc.scalar.copy(sh[0:1, o:o + 256], nat[0:1, o:o + 256])

        def blk(t):
            return t[:].rearrange("p (blk w) -> p blk w", w=W)

        eqx = btile("eqx")
        nc.vector.tensor_tensor(blk(eqx)[:, :, 1:], blk(semn)[:, :, 1:],
                                blk(semn)[:, :, :W - 1], op=OP.is_equal)
        nc.gpsimd.memset(blk(eqx)[:, :, 0:1], 1.0)
        eqy = btile("eqy")
        nc.vector.tensor_tensor(eqy[:], semn[:], sems[:], op=OP.is_equal)

        mgx = btile("mgx")
        nc.gpsimd.tensor_tensor(blk(mgx)[:, :, 1:], blk(m0n)[:, :, 1:],
                                blk(m0n)[:, :, :W - 1], op=OP.subtract)
        nc.gpsimd.memset(blk(mgx)[:, :, 0:1], 0.0)
        mgy = btile("mgy")
        nc.gpsimd.tensor_tensor(mgy[:], m0n[:], m0s[:], op=OP.subtract)
        sqmx = btile("sqmx")
        nc.scalar.activation(sqmx[:], mgx[:], AF.Square, bias=zerob[:, 0:1])
        sqmy = btile("sqmy")
        nc.scalar.activation(sqmy[:], mgy[:], AF.Square, bias=zerob[:, 0:1])
        sqmm = btile("sqmm")
        nc.vector.tensor_tensor(sqmm[:], sqmx[:], sqmy[:], op=OP.max)

        dgx = btile("dgx")
        nc.gpsimd.tensor_tensor(blk(dgx)[:, :, 1:], blk(dn)[:, :, 1:],
                                blk(dn)[:, :, :W - 1], op=OP.subtract)
        nc.gpsimd.memset(blk(dgx)[:, :, 0:1], 0.0)
        dgy = btile("dgy")
        nc.gpsimd.tensor_tensor(dgy[:], dn[:], ds_[:], op=OP.subtract)
        sqx = btile("sqx")
        nc.scalar.activation(sqx[:], dgx[:], AF.Square, bias=zerob[:, 0:1])
        sqy = btile("sqy")
        nc.scalar.activation(sqy[:], dgy[:], AF.Square, bias=zerob[:, 0:1])
        s2 = btile("s2")
        nc.vector.tensor_tensor(s2[:], sqx[:], sqy[:], op=OP.add)
        db = btile("db")
        nc.scalar.activation(db[:], s2[:], AF.Sqrt, bias=bias_sq[:, 0:1])
        u3 = btile("u3")
        nc.scalar.activation(u3[:], db[:], AF.Square, bias=zerob[:, 0:1],
                             scale=float(np.sqrt(3.0)))
        v = btile("v")
        nc.gpsimd.tensor_tensor(v[:], db[:], u3[:], op=OP.add)

        nb = btile("nb")
        nc.vector.tensor_tensor(nb[:], eqx[:], eqy[:], op=OP.mult)
        ib = btile("ib")
        nc.vector.tensor_scalar(ib[:], sqmm[:], 0.09, None, op0=OP.is_gt)
        nbib = btile("nbib")
        nc.gpsimd.tensor_tensor(nbib[:], nb[:], ib[:], op=OP.mult)
        nbv = btile("nbv")
        nc.gpsimd.tensor_tensor(nbv[:], nb[:], v[:], op=OP.mult)

        # ---- batch 1 histogram ----
        if do_hist:
            emit_hist_batch(1)

        # stats cols per batch: 5b+0 S_ib, +1 S_nb, +2 S_nbib, +3 S_nbv, +4 ent
        for b in range(BPC):
            h = slice(b * 512, (b + 1) * 512)
            cc = 5 * b
            nc.vector.tensor_reduce(stats[:, cc + 0:cc + 1], ib[:, h],
                                    axis=AX.X, op=OP.add)
            nc.vector.tensor_reduce(stats[:, cc + 1:cc + 2], nb[:, h],
                                    axis=AX.X, op=OP.add)
            nc.vector.tensor_reduce(stats[:, cc + 2:cc + 3], nbib[:, h],
                                    axis=AX.X, op=OP.add)
            nc.vector.tensor_reduce(stats[:, cc + 3:cc + 4], nbv[:, h],
                                    axis=AX.X, op=OP.add)

        if do_hist:
            emit_entropy_epilogue(1)

        # ---- cross-partition fold + output ----
        red_ps = pps.tile([1, NSTAT], f32, tag="red")
        nc.tensor.matmul(red_ps[:], ones[:], stats[:], start=True, stop=True)
        red = pconst.tile([1, NSTAT], f32, tag="redsb")
        nc.scalar.copy(red[:], red_ps[:])
        nc.sync.dma_start(out_d.ap().rearrange("(a b) -> a b", a=1), red[:])

    nc.compile()
    return nc


def _get_nc():
    global _compiled
    if _compiled is None:
        _compiled = _build()
    return _compiled


def _combine(stats):
    """stats: (NCORES, 10) -> (l_uniform, l_boundary, l_dbc, total) fp32."""
    per_b = stats.reshape(B, 5).astype(np.float64)
    s_ib, s_nb, s_nbib, s_nbv, ent = per_b.T
    inter = s_ib - s_nbib
    union = float(N) - s_nb + s_nbib + 1e-8
    l_boundary = 1.0 - np.mean(inter / union)
    l_uniform = ent.sum() / (B * M + 1e-8)
    l_dbc = s_nbv.sum() / (B * N)
    total = 0.3 * l_uniform + 0.2 * l_boundary + 0.2 * l_dbc
    return (np.float32(l_uniform), np.float32(l_boundary),
            np.float32(l_dbc), np.float32(total))


def kernel(semantic_pred, instance_masks, depth, spatial_h=H, spatial_w=W):
    global LAST_EXEC_NS
    from concourse.bass_utils import run_bass_kernel_spmd

    sem = np.ascontiguousarray(np.asarray(semantic_pred, dtype=np.int32))
    masks = np.ascontiguousarray(np.asarray(instance_masks, dtype=np.float32))
    dep = np.ascontiguousarray(np.asarray(depth, dtype=np.float32))

    nc = _get_nc()
    in_maps = [
        {"sem": sem[c * BPC:(c + 1) * BPC],
         "masks": masks[c * BPC:(c + 1) * BPC],
         "depth": dep[c * BPC:(c + 1) * BPC]}
        for c in range(NCORES)
    ]
    trace = bool(int(os.environ.get("KERNEL_TRACE", "0")))
    res = run_bass_kernel_spmd(nc, in_maps, list(range(NCORES)), trace=trace)
    LAST_EXEC_NS = res.exec_time_ns
    stats = np.stack([res.results[c]["partials"] for c in range(NCORES)])
    return _combine(stats)



# revision 1
# speedup vs baseline: 1.0240x; 1.0240x over previous
"""Trainium2 Bass kernel for the ConsistencyLoss problem (v4).

Inputs: semantic_pred (B,N) int32, instance_masks (B,M,N) f32, depth (B,N) f32
with B=16, M=32, N=65536 (H=W=256), C=27 classes. Outputs the scalar tuple
(l_uniform, l_boundary, l_dbc, total).

Sharding: pure data-parallel over batch; 2 batches per core on 8 cores. Each
core emits 10 partial sums (5 per batch); the host combines the 4 scalars.

Per-core pipeline (per batch):
  * histogram: masks stream in as [128=(g,m), 2048] f32 tiles; each tile is
    four [32, 2048] quarter-DMAs spread over sync/scalar/vector/gpsimd so
    both HWDGE rings plus SWDGE queues run in parallel. Tiles are converted
    to bf16 on ACT, then pair-transposed on the DVE by bitcasting bf16
    pairs to u32 (halves DVE transpose cost). Each 64-col stationary block
    covers 256 pixels; one matmul per block with a packed even/odd one-hot
    moving operand (N=54) accumulates an interleaved histogram in PSUM.
    Constant parity-selection matmuls fold the PSUM into hist[m, c].
  * the one-hot is built densely in one DVE 2x tensor_tensor per tile from
    the pair-transposed sem tile: m2[p, c*64+2kb+e] = (semT[p,2kb+e] == c).
  * boundary/depth losses: (256,256) views as [128, (2,256)] tiles; shifted
    rows derived on-chip via SBUF-to-SBUF DMA; emitted between the two
    batch streams so the work fills engine gaps; cross-partition fold via
    a ones matmul.
"""

import os

os.environ.setdefault("MYCRO_LOCAL_CACHE", "1")

import numpy as np
from contextlib import ExitStack

B, M, N, C = 16, 32, 65536, 27
H = W = 256
NCORES = 8
BPC = B // NCORES          # batches per core
ST = 8                     # super-tiles per batch
G = 4                      # mask-partition groups of 32
FS = 2048                  # pixels per group per super-tile (8192/tile)
KB = 32                    # stationary k-blocks per tile (256 px each)
NSTAT = 10                 # 5 partial sums x 2 batches

LAST_EXEC_NS = None

_compiled = None


def _build():
    import concourse.tile as tile
    from concourse import bacc, mybir

    f32 = mybir.dt.float32
    i32 = mybir.dt.int32
    bf16 = mybir.dt.bfloat16
    OP = mybir.AluOpType
    AX = mybir.AxisListType
    AF = mybir.ActivationFunctionType

    nc = bacc.Bacc("TRN2", target_bir_lowering=False, debug=False,
                   enable_asserts=False, num_swdge_queues=4)
    sem_d = nc.dram_tensor("sem", [BPC, N], i32, kind="ExternalInput")
    masks_d = nc.dram_tensor("masks", [BPC, M, N], f32, kind="ExternalInput")
    depth_d = nc.dram_tensor("depth", [BPC, N], f32, kind="ExternalInput")
    out_d = nc.dram_tensor("partials", [NSTAT], f32, kind="ExternalOutput")

    with tile.TileContext(nc) as tc, ExitStack() as ctx:
        pconst = ctx.enter_context(tc.tile_pool(name="const", bufs=1))
        pin = ctx.enter_context(tc.tile_pool(name="maskin", bufs=4))
        pinb = ctx.enter_context(tc.tile_pool(name="maskinb", bufs=3))
        ptr = ctx.enter_context(tc.tile_pool(name="maskT", bufs=3))
        poh = ctx.enter_context(tc.tile_pool(name="ohp", bufs=3))
        psem = ctx.enter_context(tc.tile_pool(name="semp", bufs=2))
        pbnd = ctx.enter_context(tc.tile_pool(name="bnd", bufs=1))
        psm = ctx.enter_context(tc.tile_pool(name="small", bufs=2))
        pps = ctx.enter_context(tc.tile_pool(name="psum", bufs=1, space="PSUM"))

        do_hist = not bool(int(os.environ.get("KERNEL_SKIP_HIST", "0")))

        # ---- sem path first: it gates the first one-hot build ----
        # s_t[p=(g,jj), st*64 + 2kb + e] = sem[st*8192 + g*2048 + 64kb + 2jj + e]
        sT = []
        for b in range(BPC):
            s_in = psem.tile([128, ST * 64], i32, tag="s_in", name="s_in")
            nc.sync.dma_start(s_in[:], sem_d.ap()[b].rearrange(
                "(st g j f) -> (g j) st f", st=ST, g=G, j=32, f=64))
            s_bf = psem.tile([128, ST * 64], bf16, tag="s_bf", name="s_bf")
            nc.scalar.copy(s_bf[:], s_in[:])
            s_t = psem.tile([128, ST * 64], bf16, tag="s_t", name="s_t")
            nc.vector.transpose(s_t[:].bitcast(i32), s_bf[:].bitcast(i32))
            sT.append(s_t)

        # one-hot compare pattern: col (c, kb, e) -> value c
        iotarep_i = pconst.tile([128, 2 * C * KB], i32, tag="iotarep_i")
        nc.gpsimd.iota(iotarep_i[:], pattern=[[1, C], [0, KB], [0, 2]],
                       base=0, channel_multiplier=0)
        iotarep = pconst.tile([128, 2 * C * KB], bf16, tag="iotarep")
        nc.scalar.copy(iotarep[:], iotarep_i[:])

        stats = pconst.tile([128, NSTAT], f32, tag="stats")
        nc.vector.memset(stats[:], 0.0)
        ones = pconst.tile([128, 1], f32, tag="ones")
        nc.vector.memset(ones[:], 1.0)
        bias_ln = pconst.tile([M, 1], f32, tag="bias_ln")
        nc.vector.memset(bias_ln[:], 1e-10)
        bias_sq = pconst.tile([128, 1], f32, tag="bias_sq")
        nc.vector.memset(bias_sq[:], 1e-24)
        zerob = pconst.tile([128, 1], f32, tag="zerob")
        nc.vector.memset(zerob[:], 0.0)

        hist_ps = [pps.tile([64, 2 * C], f32, tag=f"hist{b}", name=f"hist{b}")
                   for b in range(BPC)]
        dma_engs = [nc.sync, nc.scalar, nc.gpsimd]

        def emit_hist_batch(b):
            for st in range(ST):
                tin = pin.tile([128, FS], f32, tag="tin", name="tin")
                src = masks_d.ap()[b].rearrange(
                    "m (st g f) -> st g m f", st=ST, g=G)
                for g in range(G):
                    dma_engs[(g + st) % 3].dma_start(
                        tin[32 * g:32 * (g + 1), :], src[st, g])
                tinb = pinb.tile([128, FS], bf16, tag="tinb", name="tinb")
                nc.scalar.copy(tinb[:], tin[:])
                tT = ptr.tile([128, FS], bf16, tag="tT", name="tT")
                nc.vector.transpose(tT[:].bitcast(i32), tinb[:].bitcast(i32))

                # one-hot: m2[p, c*64 + 2kb + e] = (s_t[p, st*64 + 2kb+e] == c)
                m2 = poh.tile([128, 2 * C * KB], bf16, tag="m2", name="m2")
                nc.vector.tensor_tensor(
                    m2[:].rearrange("p (c kb e) -> p c kb e", c=C, kb=KB),
                    sT[b][:, st * 64:(st + 1) * 64]
                        .rearrange("p (kb e) -> p kb e", kb=KB)
                        .unsqueeze(1).broadcast_to([128, C, KB, 2]),
                    iotarep[:].rearrange("p (c kb e) -> p c kb e", c=C, kb=KB),
                    op=OP.is_equal)

                mov = m2[:].rearrange("p (c kb e) -> p kb c e", c=C, kb=KB)
                for kb in range(KB):
                    nc.tensor.matmul(
                        hist_ps[b][:],
                        tT[:, 64 * kb:64 * (kb + 1)],
                        mov[:, kb],
                        start=(st == 0 and kb == 0),
                        stop=(st == ST - 1 and kb == KB - 1),
                    )

        def emit_entropy_epilogue(b):
            # hist[m, c] = psum[2m, 2c] + psum[2m+1, 2c+1]
            psum_sb = psm.tile([64, 2 * C], f32, tag="psum_sb", name="psum_sb")
            nc.scalar.copy(psum_sb[:], hist_ps[b][:])
            psv = psum_sb[:].rearrange("p (c e) -> p e c", e=2)
            h_ps = pps.tile([M, C], f32, tag=f"hps{b}", name=f"hps{b}")
            nc.tensor.matmul(h_ps[:], sele[:], psv[:, 0], start=True, stop=False)
            nc.tensor.matmul(h_ps[:], selo[:], psv[:, 1], start=False, stop=True)
            hist = psm.tile([M, C], f32, tag="hist_sb", name="hist_sb")
            nc.scalar.copy(hist[:], h_ps[:])
            ms0 = psm.tile([M, 1], f32, tag="ms0", name="ms0")
            nc.vector.tensor_reduce(ms0[:], hist[:], axis=AX.X, op=OP.add)
            ms = psm.tile([M, 1], f32, tag="ms", name="ms")
            nc.vector.tensor_scalar(ms[:], ms0[:], 1e-6, None, op0=OP.add)
            rec = psm.tile([M, 1], f32, tag="rec", name="rec")
            nc.vector.reciprocal(rec[:], ms[:])
            pr = psm.tile([M, C], f32, tag="pr", name="pr")
            nc.vector.tensor_scalar(pr[:], hist[:], rec[:, 0:1], None, op0=OP.mult)
            ql = psm.tile([M, C], f32, tag="ql", name="ql")
            nc.scalar.activation(ql[:], pr[:], AF.Ln, bias=bias_ln[0:M, 0:1])
            escr = psm.tile([M, C], f32, tag="escr", name="escr")
            nc.vector.tensor_tensor(escr[:], pr[:], ql[:], op=OP.mult)
            ent = psm.tile([M, 1], f32, tag="ent", name="ent")
            nc.vector.tensor_reduce(ent[:], escr[:], axis=AX.X, op=OP.add)
            nc.vector.tensor_scalar(stats[0:M, 5 * b + 4:5 * b + 5], ent[:],
                                    -1.0, None, op0=OP.mult)

        # ---- batch 0 histogram ----
        if do_hist:
            emit_hist_batch(0)

        # parity-selection matrices: sele[p, m] = (p == 2m), selo: (p == 2m+1)
        selrow = pconst.tile([64, M], i32, tag="selrow")
        nc.gpsimd.iota(selrow[:], pattern=[[0, M]], base=0, channel_multiplier=1)
        selc2 = pconst.tile([64, M], i32, tag="selc2")
        nc.gpsimd.iota(selc2[:], pattern=[[2, M]], base=0, channel_multiplier=0)
        selc2p = pconst.tile([64, M], i32, tag="selc2p")
        nc.gpsimd.iota(selc2p[:], pattern=[[2, M]], base=1, channel_multiplier=0)
        sele = pconst.tile([64, M], f32, tag="sele")
        nc.vector.tensor_tensor(sele[:], selrow[:], selc2[:], op=OP.is_equal)
        selo = pconst.tile([64, M], f32, tag="selo")
        nc.vector.tensor_tensor(selo[:], selrow[:], selc2p[:], op=OP.is_equal)

        if do_hist:
            emit_entropy_epilogue(0)

        # ---- boundary + depth losses (fills gaps during batch 1) ----
        def btile(tag, dt=f32):
            return pbnd.tile([128, BPC * 512], dt, tag=tag, name=tag)

        semn = btile("semn", i32)
        sems = btile("sems", i32)
        m0n = btile("m0n")
        m0s = btile("m0s")
        dn = btile("dn")
        ds_ = btile("ds_")

        for b in range(BPC):
            o = b * 512
            img = dict(t=2, p=128, w=W)
            nc.sync.dma_start(semn[:, o:o + 512],
                              sem_d.ap()[b].rearrange("(t p w) -> p t w", **img))
            nc.scalar.dma_start(m0n[:, o:o + 512],
                                masks_d.ap()[b, 0].rearrange("(t p w) -> p t w", **img))
            nc.sync.dma_start(dn[:, o:o + 512],
                              depth_d.ap()[b].rearrange("(t p w) -> p t w", **img))

        # shifted-by-one-image-row tiles, derived on-chip
        for nat, sh in ((semn, sems), (m0n, m0s), (dn, ds_)):
            for b in range(BPC):
                o = b * 512
                nc.sync.dma_start(sh[1:128, o:o + 512], nat[0:127, o:o + 512])
                nc.sync.dma_start(sh[0:1, o + 256:o + 512], nat[127:128, o:o + 256])
                nc.scalar.copy(sh[0:1, o:o + 256], nat[0:1, o:o + 256])

        def blk(t):
            return t[:].rearrange("p (blk w) -> p blk w", w=W)

        eqx = btile("eqx")
        nc.vector.tensor_tensor(blk(eqx)[:, :, 1:], blk(semn)[:, :, 1:],
                                blk(semn)[:, :, :W - 1], op=OP.is_equal)
        nc.gpsimd.memset(blk(eqx)[:, :, 0:1], 1.0)
        eqy = btile("eqy")
        nc.vector.tensor_tensor(eqy[:], semn[:], sems[:], op=OP.is_equal)

        mgx = btile("mgx")
        nc.gpsimd.tensor_tensor(blk(mgx)[:, :, 1:], blk(m0n)[:, :, 1:],
                                blk(m0n)[:, :, :W - 1], op=OP.subtract)
        nc.gpsimd.memset(blk(mgx)[:, :, 0:1], 0.0)
        mgy = btile("mgy")
        nc.gpsimd.tensor_tensor(mgy[:], m0n[:], m0s[:], op=OP.subtract)
        sqmx = btile("sqmx")
        nc.scalar.activation(sqmx[:], mgx[:], AF.Square, bias=zerob[:, 0:1])
        sqmy = btile("sqmy")
        nc.scalar.activation(sqmy[:], mgy[:], AF.Square, bias=zerob[:, 0:1])
        sqmm = btile("sqmm")
        nc.vector.tensor_tensor(sqmm[:], sqmx[:], sqmy[:], op=OP.max)

        dgx = btile("dgx")
        nc.gpsimd.tensor_tensor(blk(dgx)[:, :, 1:], blk(dn)[:, :, 1:],
                                blk(dn)[:, :, :W - 1], op=OP.subtract)
        nc.gpsimd.memset(blk(dgx)[:, :, 0:1], 0.0)
        dgy = btile("dgy")
        nc.gpsimd.tensor_tensor(dgy[:], dn[:], ds_[:], op=OP.subtract)
        sqx = btile("sqx")
        nc.scalar.activation(sqx[:], dgx[:], AF.Square, bias=zerob[:, 0:1])
        sqy = btile("sqy")
        nc.scalar.activation(sqy[:], dgy[:], AF.Square, bias=zerob[:, 0:1])
        s2 = btile("s2")
        nc.vector.tensor_tensor(s2[:], sqx[:], sqy[:], op=OP.add)
        db = btile("db")
        nc.scalar.activation(db[:], s2[:], AF.Sqrt, bias=bias_sq[:, 0:1])
        u3 = btile("u3")
        nc.scalar.activation(u3[:], db[:], AF.Square, bias=zerob[:, 0:1],
                             scale=float(np.sqrt(3.0)))
        v = btile("v")
        nc.gpsimd.tensor_tensor(v[:], db[:], u3[:], op=OP.add)

        nb = btile("nb")
        nc.vector.tensor_tensor(nb[:], eqx[:], eqy[:], op=OP.mult)
        ib = btile("ib")
        nc.vector.tensor_scalar(ib[:], sqmm[:], 0.09, None, op0=OP.is_gt)
        nbib = btile("nbib")
        nc.gpsimd.tensor_tensor(nbib[:], nb[:], ib[:], op=OP.mult)
        nbv = btile("nbv")
        nc.gpsimd.tensor_tensor(nbv[:], nb[:], v[:], op=OP.mult)

        # ---- batch 1 histogram ----
        if do_hist:
            emit_hist_batch(1)

        # stats cols per batch: 5b+0 S_ib, +1 S_nb, +2 S_nbib, +3 S_nbv, +4 ent
        for b in range(BPC):
            h = slice(b * 512, (b + 1) * 512)
            cc = 5 * b
            nc.vector.tensor_reduce(stats[:, cc + 0:cc + 1], ib[:, h],
                                    axis=AX.X, op=OP.add)
            nc.vector.tensor_reduce(stats[:, cc + 1:cc + 2], nb[:, h],
                                    axis=AX.X, op=OP.add)
            nc.vector.tensor_reduce(stats[:, cc + 2:cc + 3], nbib[:, h],
                                    axis=AX.X, op=OP.add)
            nc.vector.tensor_reduce(stats[:, cc + 3:cc + 4], nbv[:, h],
                                    axis=AX.X, op=OP.add)

        if do_hist:
            emit_entropy_epilogue(1)

        # ---- cross-partition fold + output ----
        red_ps = pps.tile([1, NSTAT], f32, tag="red")
        nc.tensor.matmul(red_ps[:], ones[:], stats[:], start=True, stop=True)
        red = pconst.tile([1, NSTAT], f32, tag="redsb")
        nc.scalar.copy(red[:], red_ps[:])
        nc.sync.dma_start(out_d.ap().rearrange("(a b) -> a b", a=1), red[:])

    nc.compile()
    return nc


def _get_nc():
    global _compiled
    if _compiled is None:
        _compiled = _build()
    return _compiled


def _combine(stats):
    """stats: (NCORES, 10) -> (l_uniform, l_boundary, l_dbc, total) fp32."""
    per_b = stats.reshape(B, 5).astype(np.float64)
    s_ib, s_nb, s_nbib, s_nbv, ent = per_b.T
    inter = s_ib - s_nbib
    union = float(N) - s_nb + s_nbib + 1e-8
    l_boundary = 1.0 - np.mean(inter / union)
    l_uniform = ent.sum() / (B * M + 1e-8)
    l_dbc = s_nbv.sum() / (B * N)
    total = 0.3 * l_uniform + 0.2 * l_boundary + 0.2 * l_dbc
    return (np.float32(l_uniform), np.float32(l_boundary),
            np.float32(l_dbc), np.float32(total))


def kernel(semantic_pred, instance_masks, depth, spatial_h=H, spatial_w=W):
    global LAST_EXEC_NS
    from concourse.bass_utils import run_bass_kernel_spmd

    sem = np.ascontiguousarray(np.asarray(semantic_pred, dtype=np.int32))
    masks = np.ascontiguousarray(np.asarray(instance_masks, dtype=np.float32))
    dep = np.ascontiguousarray(np.asarray(depth, dtype=np.float32))

    nc = _get_nc()
    in_maps = [
        {"sem": sem[c * BPC:(c + 1) * BPC],
         "masks": masks[c * BPC:(c + 1) * BPC],
         "depth": dep[c * BPC:(c + 1) * BPC]}
        for c in range(NCORES)
    ]
    trace = bool(int(os.environ.get("KERNEL_TRACE", "0")))
    res = run_bass_kernel_spmd(nc, in_maps, list(range(NCORES)), trace=trace)
    LAST_EXEC_NS = res.exec_time_ns
    stats = np.stack([res.results[c]["partials"] for c in range(NCORES)])
    return _combine(stats)

